# revision 26
# baseline (speedup 1.0000x reference)
"""Multi-head cross-attention on 8 Trainium2 NeuronCores.

Sharding: core = (batch b, T-half). Each core computes the full output slab
out[b, t0:t0+512, :] locally: q projection for its rows, k/v projection for
its batch (duplicated across the 2 cores sharing a batch), attention for all
16 heads, and the output projection. No collectives.

v2 design (all-bf16 matmul pipeline, fp32 PSUM accumulate):
  - x, y are transposed AND cast to bf16 on the host, so xT/yT tiles DMA
    straight into SBUF: no stage tiles, no PE transposes, no DVE copies.
    Weights are also host-cast to bf16: every pool fits in SBUF at once, so
    no WAR serialization between projection phases.
  - qT[hd, T]  = Wq.T chunks @ xT          (bf16, weights stationary)
  - kT pair j  -> kt0/kt1 [128, S] bf16, the sibling head's 64 rows ZEROED
    (memset on the otherwise-idle Pool engine): scores run as full-grid
    K=128 matmuls, keeping the back-to-back pitch.
  - scoresT + exp(scale*s) emitted right after each kT chunk so ACT's exp
    work hides under the qkv matmuls (no max-subtraction needed:
    |s*scale| < ~2, exp in [e^-2, e^2])
  - v[S, h, 65] bf16 (col 64 = ones -> AV row 64 = softmax denominators)
  - av[65, T] = [v_h | 1].T @ expT          (bf16, full-grid)
  - AV PSUM rows staged into per-pair [128, T] f32 tiles by the Pool
    engine; denominators parked at contiguous partitions 2j/2j+1 of one
    tile; per 2 pairs one reciprocal_approx_fast (DVE, ~5x faster than
    InstReciprocal which stalled the PE ~4us per group in v1); 1/den
    broadcast via one K=2 selector matmul per pair; at = avs * bcast (DVE)
  - out[T, C] = AT.T-chunks @ WoT + ones.T @ bo   (bf16; bias via K=1 matmul)
"""

import numpy as np
from contextlib import ExitStack

import ml_dtypes

import concourse.bass as bass
import concourse.bacc as bacc
import concourse.mybir as mybir
import concourse.tile as tile
from concourse.bass_utils import run_bass_kernel_spmd

F32 = mybir.dt.float32
F32R = mybir.dt.float32r
BF16 = mybir.dt.bfloat16
EXP = mybir.ActivationFunctionType.Exp
COPY = mybir.ActivationFunctionType.Copy

# Full problem constants (nn_MultiHeadCrossAttention: B,T,S,C,H,D)
B_FULL, T_FULL, S_FULL, C_FULL, H_FULL, D_FULL = 4, 1024, 1024, 1024, 16, 64
N_CORES = 8

BF16_NP = ml_dtypes.bfloat16


def build_attention_nc(T=512, S=1024, C=1024, H=16, D=64, scale=None,
                       et_bufs=40, debug_taps=False):
    """Per-core kernel. T rows of queries, S source rows, all H heads."""
    assert T % 128 == 0 and S % 128 == 0 and C % 128 == 0 and D == 64
    assert T <= 512  # scores/AV moving-operand free size (one PSUM bank)
    if scale is None:
        scale = C ** (-0.5)
    HD = H * D
    CK, SK, TK, MK = C // 128, S // 128, T // 128, HD // 128

    def nchunks(total):
        return [(i, min(512, total - i)) for i in range(0, total, 512)]

    nc = bacc.Bacc("TRN2", target_bir_lowering=False, debug=False,
                   num_devices=N_CORES)
    xt_dr = nc.dram_tensor("xt", [C, T], BF16, kind="ExternalInput")
    yt_dr = nc.dram_tensor("yt", [C, S], BF16, kind="ExternalInput")
    wq = nc.dram_tensor("wq", [C, HD], BF16, kind="ExternalInput")
    wk = nc.dram_tensor("wk", [C, HD], BF16, kind="ExternalInput")
    wv = nc.dram_tensor("wv", [C, HD], BF16, kind="ExternalInput")
    wot = nc.dram_tensor("wot", [HD, C], BF16, kind="ExternalInput")
    bo = nc.dram_tensor("bo", [1, C], BF16, kind="ExternalInput")
    o = nc.dram_tensor("o", [T, C], F32, kind="ExternalOutput")
    taps = {}
    if debug_taps:
        taps = {
            "qt0_d": nc.dram_tensor("qt0_d", [128, T], BF16,
                                    kind="ExternalOutput"),
            "kt00_d": nc.dram_tensor("kt00_d", [128, S], BF16,
                                     kind="ExternalOutput"),
            "v0_d": nc.dram_tensor("v0_d", [128, H * 65], BF16,
                                   kind="ExternalOutput"),
            "at0_d": nc.dram_tensor("at0_d", [128, T], BF16,
                                    kind="ExternalOutput"),
            "xt0_d": nc.dram_tensor("xt0_d", [128, T], BF16,
                                    kind="ExternalOutput"),
        }

    # SBUF pools are a two-sided stack allocator: open/close must be LIFO
    # per side.  Left: pers > yt > wk > (xt > wq) > (at, pair, osb).
    # Right: et > wot > wv.
    with tile.TileContext(nc) as tc, ExitStack() as ctx:
        pers = ctx.enter_context(tc.tile_pool(name="pers", bufs=1))

        qt = [pers.tile([128, T], BF16, tag=f"qt{m}", name=f"qt{m}")
              for m in range(MK)]
        kt0 = [pers.tile([128, S], BF16, tag=f"kt0_{m}", name=f"kt0_{m}")
               for m in range(MK)]
        kt1 = [pers.tile([128, S], BF16, tag=f"kt1_{m}", name=f"kt1_{m}")
               for m in range(MK)]
        v_sb = [pers.tile([128, H, 65], BF16, tag=f"v{s}", name=f"v{s}")
                for s in range(SK)]
        # 1/den staging ring: per pair, reciprocal rows land at partitions 0
        # and 32 (engine APs must start at a multiple-of-32 partition; the
        # two K=1 broadcast matmuls then need matching stationary/moving
        # bases, hence the ones rows at partitions 0 and 32)
        p_norm = ctx.enter_context(tc.tile_pool(name="normpool", bufs=2))
        ones1 = pers.tile([33, 128], BF16, tag="ones1", name="ones1")
        bo_sb = pers.tile([1, C], BF16, tag="bo", name="bo_sb")

        # constants + zero-padding memsets all go to the idle Pool engine,
        # during the DMA warmup window
        for b in (0, 32):
            nc.gpsimd.memset(ones1[b:b + 1, :], 1.0)
        for m in range(MK):
            nc.gpsimd.memset(kt0[m][64:128, :], 0.0)
            nc.gpsimd.memset(kt1[m][0:64, :], 0.0)
        for s in range(SK):
            nc.gpsimd.memset(v_sb[s][:, :, 64:65], 1.0)

        s_yt, s_wk = ExitStack(), ExitStack()
        s_xt, s_wq = ExitStack(), ExitStack()
        s_et, s_wot, s_wv = ExitStack(), ExitStack(), ExitStack()
        s_psqk, s_pss, s_psv, s_psav = (ExitStack(), ExitStack(),
                                        ExitStack(), ExitStack())
        try:
            # ---- pools (everything co-resident; LIFO per side) ------------
            p_yt = s_yt.enter_context(tc.tile_pool(name="ytp", bufs=1))
            yt = [p_yt.tile([128, S], BF16, tag=f"yt{k}", name=f"ytt{k}")
                  for k in range(CK)]
            p_wk = s_wk.enter_context(tc.tile_pool(name="wkp", bufs=1))
            wk_sb = [p_wk.tile([128, HD], BF16, tag=f"wk{k}", name=f"wk{k}")
                     for k in range(CK)]
            p_xt = s_xt.enter_context(tc.tile_pool(name="xtp", bufs=1))
            xt = [p_xt.tile([128, T], BF16, tag=f"xt{k}", name=f"xtt{k}")
                  for k in range(CK)]
            p_wq = s_wq.enter_context(tc.tile_pool(name="wqp", bufs=1))
            wq_sb = [p_wq.tile([128, HD], BF16, tag=f"wq{k}", name=f"wq{k}")
                     for k in range(CK)]
            p_et = s_et.enter_context(
                tc.tile_pool(name="etpool", bufs=et_bufs, side="right"))
            p_wot = s_wot.enter_context(
                tc.tile_pool(name="wotp", bufs=1, side="right"))
            wot_sb = [p_wot.tile([128, C], BF16, tag=f"wot{k}",
                                 name=f"wot{k}") for k in range(MK)]
            p_wv = s_wv.enter_context(
                tc.tile_pool(name="wvp", bufs=1, side="right"))
            wv_sb = [p_wv.tile([128, HD], BF16, tag=f"wv{k}", name=f"wv{k}")
                     for k in range(CK)]

            # ---- DMA issue, in order of first use -------------------------
            # 64KB chunks so the DMA queues fill the first tiles fast;
            # ordered (xt, wq-half0) -> yt -> wq-half1 -> wk -> wv -> wot so
            # each phase's operands land just before the PE needs them
            def wchunks(tiles, dram, lo, hi, step=256):
                for k in range(len(tiles)):
                    for c0 in range(lo, hi, step):
                        nc.sync.dma_start(
                            out=tiles[k][:, c0:c0 + step],
                            in_=dram[k * 128:(k + 1) * 128, c0:c0 + step])

            wchunks(xt, xt_dr, 0, T)
            wchunks(wq_sb, wq, 0, HD // 2)
            wchunks(yt, yt_dr, 0, S)
            wchunks(wq_sb, wq, HD // 2, HD)
            wchunks(wk_sb, wk, 0, HD)
            wchunks(wv_sb, wv, 0, HD)
            nc.sync.dma_start(out=bo_sb, in_=bo[:, :])
            wchunks(wot_sb, wot, 0, C)

            # ---- phase B: qT, kT, scoresT + exp ---------------------------
            # PSUM is also a two-sided LIFO stack: ps_s (right) outlives the
            # per-phase accumulator pools (left) but closes before ps_o
            ps_s = s_pss.enter_context(
                tc.tile_pool(name="ps_s", bufs=4, space="PSUM", side="right"))
            ps_qk = s_psqk.enter_context(
                tc.tile_pool(name="ps_qk", bufs=4, space="PSUM"))

            for m in range(MK):
                psq = ps_qk.tile([128, T], F32, tag="psqk", name="psq")
                for k in range(CK):
                    nc.tensor.matmul(
                        psq[:], wq_sb[k][:, m * 128:(m + 1) * 128],
                        xt[k][:], start=(k == 0), stop=(k == CK - 1))
                # ACT is idle until the first scores land; qt casts go there
                nc.scalar.activation(out=qt[m][:], in_=psq[:], func=COPY)

            ets = {}

            def emit_scores(j):
                """scoresT + exp for head pair j (full-grid K=128 matmuls
                against zero-padded kt0/kt1)."""
                et0, et1 = [], []
                for s in range(SK):
                    pss0 = ps_s.tile([128, T], F32, tag="pss", name="pss")
                    nc.tensor.matmul(
                        pss0[:], kt0[j][:, s * 128:(s + 1) * 128],
                        qt[j][:, :], start=True, stop=True)
                    pss1 = ps_s.tile([128, T], F32, tag="pss", name="pss")
                    nc.tensor.matmul(
                        pss1[:], kt1[j][:, s * 128:(s + 1) * 128],
                        qt[j][:, :], start=True, stop=True)
                    e0 = p_et.tile([128, T], BF16, tag="et", name="et")
                    nc.scalar.activation(out=e0[:], in_=pss0[:], func=EXP,
                                         scale=float(scale))
                    e1 = p_et.tile([128, T], BF16, tag="et", name="et")
                    nc.scalar.activation(out=e1[:], in_=pss1[:], func=EXP,
                                         scale=float(scale))
                    et0.append(e0)
                    et1.append(e1)
                ets[j] = (et0, et1)

            for m in range(MK):
                for off, sz in nchunks(S):
                    psk = ps_qk.tile([128, sz], F32, tag="psqk", name="psk")
                    for k in range(CK):
                        nc.tensor.matmul(
                            psk[:], wk_sb[k][:, m * 128:(m + 1) * 128],
                            yt[k][:, off:off + sz],
                            start=(k == 0), stop=(k == CK - 1))
                    nc.vector.tensor_copy(kt0[m][0:64, off:off + sz],
                                          psk[0:64, :])
                    nc.vector.tensor_copy(kt1[m][64:128, off:off + sz],
                                          psk[64:128, :])
                emit_scores(m)
            if debug_taps:
                nc.sync.dma_start(out=taps["xt0_d"][:, :], in_=xt[0][:])
            s_wq.close()
            s_xt.close()
            s_wk.close()
            s_psqk.close()

            # ---- phase C: v natural + ones column -------------------------
            ps_v = s_psv.enter_context(
                tc.tile_pool(name="ps_v", bufs=4, space="PSUM"))
            for s in range(SK):
                for off, sz in nchunks(HD):
                    psv = ps_v.tile([128, sz], F32, tag="psv", name="psv")
                    for k in range(CK):
                        nc.tensor.matmul(
                            psv[:], yt[k][:, s * 128:(s + 1) * 128],
                            wv_sb[k][:, off:off + sz],
                            start=(k == 0), stop=(k == CK - 1))
                    h0 = off // 64
                    nc.vector.tensor_copy(
                        v_sb[s][:, h0:h0 + sz // 64, 0:64],
                        psv[:].rearrange("p (h d) -> p h d", d=64))
            s_wv.close()
            s_yt.close()
            s_psv.close()

            # ---- phase D: attention per head pair -------------------------
            ps_av = s_psav.enter_context(
                tc.tile_pool(name="ps_av", bufs=1, space="PSUM"))

            with ExitStack() as ctx_d:
                p_at = ctx_d.enter_context(tc.tile_pool(name="atpool", bufs=1))
                at = [p_at.tile([128, T], BF16, tag=f"at{m}", name=f"at{m}")
                      for m in range(MK)]
                p_pair = ctx_d.enter_context(
                    tc.tile_pool(name="pairpool", bufs=4))
                pairbuf = {}

                def emit_av(j):
                    """AV for pair j — 16 back-to-back full-row matmuls; DVE
                    stages AV rows + denominator rows (partitions 0/32 of a
                    per-pair tile — matmul-legal bases) out of PSUM."""
                    et0, et1 = ets[j]
                    denb = p_norm.tile([33, T], F32, tag="den", bufs=3,
                                       name="denb")
                    pairT = p_pair.tile([128, T], F32, tag="pair", bufs=4,
                                        name="pairT")
                    psav0 = ps_av.tile([65, T], F32, tag="psav", bufs=2,
                                       name="psav")
                    for s in range(SK):
                        nc.tensor.matmul(psav0[:], v_sb[s][:, 2 * j, 0:65],
                                         et0[s][:],
                                         start=(s == 0), stop=(s == SK - 1))
                    nc.vector.tensor_copy(pairT[0:64, :], psav0[0:64, :])
                    nc.vector.tensor_copy(denb[0:1, :], psav0[64:65, :])
                    psav1 = ps_av.tile([65, T], F32, tag="psav", bufs=2,
                                       name="psav")
                    for s in range(SK):
                        nc.tensor.matmul(psav1[:], v_sb[s][:, 2 * j + 1, 0:65],
                                         et1[s][:],
                                         start=(s == 0), stop=(s == SK - 1))
                    nc.vector.tensor_copy(pairT[64:128, :], psav1[0:64, :])
                    nc.vector.tensor_copy(denb[32:33, :], psav1[64:65, :])
                    pairbuf[j] = (pairT, denb)

                def emit_norm(j):
                    """Normalize pair j: one fast reciprocal + one bf16 cast
                    batched over both denominator rows, two K=1 broadcast
                    matmuls, one [128,T] multiply."""
                    pairT, denb = pairbuf.pop(j)
                    recf = p_norm.tile([33, T], F32, tag="recf", bufs=2,
                                       name="recf")
                    rec = p_norm.tile([33, T], BF16, tag="rec", bufs=2,
                                      name="rec")
                    # batched over both parked rows; the unwritten rows in
                    # between produce garbage that is never read
                    nc.vector.reciprocal_approx_fast(
                        out=recf[0:33, :], in_=denb[0:33, :])
                    nc.vector.tensor_copy(rec[0:33, :], recf[0:33, :])
                    psb = ps_av.tile([128, T], F32, tag="psb", bufs=2,
                                     name="psb")
                    nc.tensor.matmul(psb[0:64, :], ones1[0:1, 0:64],
                                     rec[0:1, :], start=True, stop=True)
                    nc.tensor.matmul(psb[64:128, :], ones1[32:33, 0:64],
                                     rec[32:33, :], start=True, stop=True)
                    with nc.allow_low_precision(reason="at is bf16"):
                        nc.vector.tensor_mul(at[j][:], pairT[:], psb[:])

                for j in range(H // 2):
                    emit_av(j)
                    if j >= 1:
                        emit_norm(j - 1)
                emit_norm(H // 2 - 1)

                if debug_taps:
                    nc.sync.dma_start(out=taps["qt0_d"][:, :], in_=qt[0][:])
                    nc.sync.dma_start(out=taps["kt00_d"][:, :], in_=kt0[0][:])
                    nc.sync.dma_start(
                        out=taps["v0_d"][:, :],
                        in_=v_sb[0][:].rearrange("p h d -> p (h d)"))
                    nc.sync.dma_start(out=taps["at0_d"][:, :], in_=at[0][:])

                # ---- phase E: output projection + bias --------------------
                s_pss.close()
                ps_o = ctx_d.enter_context(
                    tc.tile_pool(name="ps_o", bufs=2, space="PSUM",
                                 side="right"))
                p_o = ctx_d.enter_context(tc.tile_pool(name="opool", bufs=2))
                for t_ in range(TK):
                    o_sb = p_o.tile([128, C], F32, tag="osb", name="o_sb")
                    for off, sz in nchunks(C):
                        pso = ps_o.tile([128, sz], F32, tag="pso", name="pso")
                        for mk in range(MK):
                            nc.tensor.matmul(
                                pso[:], at[mk][:, t_ * 128:(t_ + 1) * 128],
                                wot_sb[mk][:, off:off + sz],
                                start=(mk == 0), stop=False)
                        nc.tensor.matmul(pso[:], ones1[0:1, 0:128],
                                         bo_sb[0:1, off:off + sz],
                                         start=False, stop=True)
                        nc.scalar.activation(out=o_sb[:, off:off + sz],
                                             in_=pso[:], func=COPY)
                    for off, sz in nchunks(C):
                        nc.sync.dma_start(
                            out=o[t_ * 128:(t_ + 1) * 128, off:off + sz],
                            in_=o_sb[:, off:off + sz])
            s_wot.close()
            s_et.close()
        finally:
            for st in (s_wq, s_xt, s_wk, s_yt, s_wv, s_wot, s_et,
                       s_psqk, s_pss, s_psv, s_psav):
                st.close()

    nc.compile()
    return nc


# ---------------------------------------------------------------------------
# Host-side wrapper
# ---------------------------------------------------------------------------

_NC_CACHE = {}


def _get_nc():
    key = "full"
    if key not in _NC_CACHE:
        _NC_CACHE[key] = build_attention_nc(
            T=T_FULL * B_FULL // N_CORES, S=S_FULL, C=C_FULL, H=H_FULL,
            D=D_FULL, scale=C_FULL ** (-0.5))
    return _NC_CACHE[key]


def make_in_maps(x, y_enc, Wq, Wk, Wv, Wo, bo):
    """Shard full inputs into the 8 per-core input maps (host prep: bf16
    cast + pre-transpose of x/y, head-stacked weight layouts)."""
    x = np.asarray(x, dtype=np.float32)
    y_enc = np.asarray(y_enc, dtype=np.float32)
    Wq = np.asarray(Wq, dtype=np.float32)
    Wk = np.asarray(Wk, dtype=np.float32)
    Wv = np.asarray(Wv, dtype=np.float32)
    Wo = np.asarray(Wo, dtype=np.float32)
    bo = np.asarray(bo, dtype=np.float32)

    C = Wq.shape[1]
    HD = Wq.shape[0] * Wq.shape[2]
    wq_p = np.ascontiguousarray(
        Wq.transpose(1, 0, 2).reshape(C, HD).astype(BF16_NP))
    wk_p = np.ascontiguousarray(
        Wk.transpose(1, 0, 2).reshape(C, HD).astype(BF16_NP))
    wv_p = np.ascontiguousarray(
        Wv.transpose(1, 0, 2).reshape(C, HD).astype(BF16_NP))
    wot = np.ascontiguousarray(Wo.T.astype(BF16_NP))
    bo2 = np.ascontiguousarray(bo.reshape(1, -1).astype(BF16_NP))

    T = x.shape[1] * x.shape[0] // N_CORES  # rows per core
    yts = [np.ascontiguousarray(y_enc[b].T.astype(BF16_NP))
           for b in range(x.shape[0])]
    in_maps = []
    for core in range(N_CORES):
        b, half = divmod(core, N_CORES // x.shape[0])
        xt = np.ascontiguousarray(
            x[b, half * T:(half + 1) * T].T.astype(BF16_NP))
        in_maps.append({
            "xt": xt, "yt": yts[b],
            "wq": wq_p, "wk": wk_p, "wv": wv_p, "wot": wot, "bo": bo2,
        })
    return in_maps


def run(inputs, trace=False, trace_cores=None):
    """Compile + run on the 8 cores; returns (out, BassKernelResults)."""
    nc = _get_nc()
    in_maps = make_in_maps(**inputs)
    kw = {}
    if trace:
        kw = dict(trace=True,
                  trace_cores=trace_cores if trace_cores is not None else [0])
    res = run_bass_kernel_spmd(nc, in_maps, core_ids=list(range(N_CORES)), **kw)

    B, T_full, C = np.asarray(inputs["x"]).shape
    T = T_full * B // N_CORES
    out = np.empty((B, T_full, C), dtype=np.float32)
    for core in range(N_CORES):
        b, half = divmod(core, N_CORES // B)
        out[b, half * T:(half + 1) * T] = res.results[core]["o"]
    return out, res


def kernel(x, y_enc, Wq, Wk, Wv, Wo, bo):
    out, _ = run(dict(x=x, y_enc=y_enc, Wq=Wq, Wk=Wk, Wv=Wv, Wo=Wo, bo=bo))
    return out


# revision 27
# speedup vs baseline: 1.0106x; 1.0106x over previous
"""Multi-head cross-attention on 8 Trainium2 NeuronCores.

Sharding: core = (batch b, T-half). Each core computes the full output slab
out[b, t0:t0+512, :] locally: q projection for its rows, k/v projection for
its batch (duplicated across the 2 cores sharing a batch), attention for all
16 heads, and the output projection. No collectives.

v2 design (all-bf16 matmul pipeline, fp32 PSUM accumulate):
  - x, y are transposed AND cast to bf16 on the host, so xT/yT tiles DMA
    straight into SBUF: no stage tiles, no PE transposes, no DVE copies.
    Weights are also host-cast to bf16: every pool fits in SBUF at once, so
    no WAR serialization between projection phases.
  - qT[hd, T]  = Wq.T chunks @ xT          (bf16, weights stationary)
  - kT pair j  -> kt0/kt1 [128, S] bf16, the sibling head's 64 rows ZEROED
    (memset on the otherwise-idle Pool engine): scores run as full-grid
    K=128 matmuls, keeping the back-to-back pitch.
  - scoresT + exp(scale*s) emitted right after each kT chunk so ACT's exp
    work hides under the qkv matmuls (no max-subtraction needed:
    |s*scale| < ~2, exp in [e^-2, e^2])
  - v[S, h, 65] bf16 (col 64 = ones -> AV row 64 = softmax denominators)
  - av[65, T] = [v_h | 1].T @ expT          (bf16, full-grid)
  - AV PSUM rows staged into per-pair [128, T] f32 tiles by the Pool
    engine; denominators parked at contiguous partitions 2j/2j+1 of one
    tile; per 2 pairs one reciprocal_approx_fast (DVE, ~5x faster than
    InstReciprocal which stalled the PE ~4us per group in v1); 1/den
    broadcast via one K=2 selector matmul per pair; at = avs * bcast (DVE)
  - out[T, C] = AT.T-chunks @ WoT + ones.T @ bo   (bf16; bias via K=1 matmul)
"""

import numpy as np
from contextlib import ExitStack

import ml_dtypes

import concourse.bass as bass
import concourse.bacc as bacc
import concourse.mybir as mybir
import concourse.tile as tile
from concourse.bass_utils import run_bass_kernel_spmd

F32 = mybir.dt.float32
F32R = mybir.dt.float32r
BF16 = mybir.dt.bfloat16
EXP = mybir.ActivationFunctionType.Exp
COPY = mybir.ActivationFunctionType.Copy

# Full problem constants (nn_MultiHeadCrossAttention: B,T,S,C,H,D)
B_FULL, T_FULL, S_FULL, C_FULL, H_FULL, D_FULL = 4, 1024, 1024, 1024, 16, 64
N_CORES = 8

BF16_NP = ml_dtypes.bfloat16


def build_attention_nc(T=512, S=1024, C=1024, H=16, D=64, scale=None,
                       et_bufs=40, debug_taps=False):
    """Per-core kernel. T rows of queries, S source rows, all H heads."""
    assert T % 128 == 0 and S % 128 == 0 and C % 128 == 0 and D == 64
    assert T <= 512  # scores/AV moving-operand free size (one PSUM bank)
    if scale is None:
        scale = C ** (-0.5)
    HD = H * D
    CK, SK, TK, MK = C // 128, S // 128, T // 128, HD // 128

    def nchunks(total):
        return [(i, min(512, total - i)) for i in range(0, total, 512)]

    nc = bacc.Bacc("TRN2", target_bir_lowering=False, debug=False,
                   num_devices=N_CORES)
    xt_dr = nc.dram_tensor("xt", [C, T], BF16, kind="ExternalInput")
    yt_dr = nc.dram_tensor("yt", [C, S], BF16, kind="ExternalInput")
    wq = nc.dram_tensor("wq", [C, HD], BF16, kind="ExternalInput")
    wk = nc.dram_tensor("wk", [C, HD], BF16, kind="ExternalInput")
    wv = nc.dram_tensor("wv", [C, HD], BF16, kind="ExternalInput")
    wot = nc.dram_tensor("wot", [HD, C], BF16, kind="ExternalInput")
    bo = nc.dram_tensor("bo", [1, C], BF16, kind="ExternalInput")
    o = nc.dram_tensor("o", [T, C], F32, kind="ExternalOutput")
    taps = {}
    if debug_taps:
        taps = {
            "qt0_d": nc.dram_tensor("qt0_d", [128, T], BF16,
                                    kind="ExternalOutput"),
            "kt00_d": nc.dram_tensor("kt00_d", [128, S], BF16,
                                     kind="ExternalOutput"),
            "v0_d": nc.dram_tensor("v0_d", [128, H * 65], BF16,
                                   kind="ExternalOutput"),
            "at0_d": nc.dram_tensor("at0_d", [128, T], BF16,
                                    kind="ExternalOutput"),
            "xt0_d": nc.dram_tensor("xt0_d", [128, T], BF16,
                                    kind="ExternalOutput"),
        }

    # SBUF pools are a two-sided stack allocator: open/close must be LIFO
    # per side.  Left: pers > yt > wk > (xt > wq) > (at, pair, osb).
    # Right: et > wot > wv.
    with tile.TileContext(nc) as tc, ExitStack() as ctx:
        pers = ctx.enter_context(tc.tile_pool(name="pers", bufs=1))

        qt = [pers.tile([128, T], BF16, tag=f"qt{m}", name=f"qt{m}")
              for m in range(MK)]
        kt0 = [pers.tile([128, S], BF16, tag=f"kt0_{m}", name=f"kt0_{m}")
               for m in range(MK)]
        kt1 = [pers.tile([128, S], BF16, tag=f"kt1_{m}", name=f"kt1_{m}")
               for m in range(MK)]
        v_sb = [pers.tile([128, H, 65], BF16, tag=f"v{s}", name=f"v{s}")
                for s in range(SK)]
        # 1/den staging ring: per pair, reciprocal rows land at partitions 0
        # and 32 (engine APs must start at a multiple-of-32 partition; the
        # two K=1 broadcast matmuls then need matching stationary/moving
        # bases, hence the ones rows at partitions 0 and 32)
        p_norm = ctx.enter_context(tc.tile_pool(name="normpool", bufs=2))
        ones1 = pers.tile([33, 128], BF16, tag="ones1", name="ones1")
        bo_sb = pers.tile([1, C], BF16, tag="bo", name="bo_sb")

        # constants + zero-padding memsets all go to the idle Pool engine,
        # during the DMA warmup window
        for b in (0, 32):
            nc.gpsimd.memset(ones1[b:b + 1, :], 1.0)
        for m in range(MK):
            nc.gpsimd.memset(kt0[m][64:128, :], 0.0)
            nc.gpsimd.memset(kt1[m][0:64, :], 0.0)
        for s in range(SK):
            nc.gpsimd.memset(v_sb[s][:, :, 64:65], 1.0)

        s_yt, s_wk = ExitStack(), ExitStack()
        s_xt, s_wq = ExitStack(), ExitStack()
        s_et, s_wot, s_wv = ExitStack(), ExitStack(), ExitStack()
        s_psqk, s_pss, s_psv, s_psav = (ExitStack(), ExitStack(),
                                        ExitStack(), ExitStack())
        try:
            # ---- pools (everything co-resident; LIFO per side) ------------
            p_yt = s_yt.enter_context(tc.tile_pool(name="ytp", bufs=1))
            yt = [p_yt.tile([128, S], BF16, tag=f"yt{k}", name=f"ytt{k}")
                  for k in range(CK)]
            p_wk = s_wk.enter_context(tc.tile_pool(name="wkp", bufs=1))
            wk_sb = [p_wk.tile([128, HD], BF16, tag=f"wk{k}", name=f"wk{k}")
                     for k in range(CK)]
            p_xt = s_xt.enter_context(tc.tile_pool(name="xtp", bufs=1))
            xt = [p_xt.tile([128, T], BF16, tag=f"xt{k}", name=f"xtt{k}")
                  for k in range(CK)]
            p_wq = s_wq.enter_context(tc.tile_pool(name="wqp", bufs=1))
            wq_sb = [p_wq.tile([128, HD], BF16, tag=f"wq{k}", name=f"wq{k}")
                     for k in range(CK)]
            p_et = s_et.enter_context(
                tc.tile_pool(name="etpool", bufs=et_bufs, side="right"))
            p_wot = s_wot.enter_context(
                tc.tile_pool(name="wotp", bufs=1, side="right"))
            wot_sb = [p_wot.tile([128, C], BF16, tag=f"wot{k}",
                                 name=f"wot{k}") for k in range(MK)]
            p_wv = s_wv.enter_context(
                tc.tile_pool(name="wvp", bufs=1, side="right"))
            wv_sb = [p_wv.tile([128, HD], BF16, tag=f"wv{k}", name=f"wv{k}")
                     for k in range(CK)]

            # ---- DMA issue, in order of first use -------------------------
            # 64KB chunks so the DMA queues fill the first tiles fast;
            # ordered (xt, wq-half0) -> yt -> wq-half1 -> wk -> wv -> wot so
            # each phase's operands land just before the PE needs them
            def wchunks(tiles, dram, lo, hi, step=512):
                for k in range(len(tiles)):
                    for c0 in range(lo, hi, step):
                        nc.sync.dma_start(
                            out=tiles[k][:, c0:c0 + step],
                            in_=dram[k * 128:(k + 1) * 128, c0:c0 + step])

            for k in range(CK):
                nc.sync.dma_start(out=xt[k],
                                  in_=xt_dr[k * 128:(k + 1) * 128, :])
            wchunks(wq_sb, wq, 0, HD // 2)
            wchunks(yt, yt_dr, 0, S)
            wchunks(wq_sb, wq, HD // 2, HD)
            wchunks(wk_sb, wk, 0, HD)
            wchunks(wv_sb, wv, 0, HD)
            nc.sync.dma_start(out=bo_sb, in_=bo[:, :])
            wchunks(wot_sb, wot, 0, C)

            # ---- phase B: qT, kT, scoresT + exp ---------------------------
            # PSUM is also a two-sided LIFO stack: ps_s (right) outlives the
            # per-phase accumulator pools (left) but closes before ps_o
            ps_s = s_pss.enter_context(
                tc.tile_pool(name="ps_s", bufs=4, space="PSUM", side="right"))
            ps_qk = s_psqk.enter_context(
                tc.tile_pool(name="ps_qk", bufs=4, space="PSUM"))

            for m in range(MK):
                psq = ps_qk.tile([128, T], F32, tag="psqk", name="psq")
                for k in range(CK):
                    nc.tensor.matmul(
                        psq[:], wq_sb[k][:, m * 128:(m + 1) * 128],
                        xt[k][:], start=(k == 0), stop=(k == CK - 1))
                # ACT is idle until the first scores land; qt casts go there
                nc.scalar.activation(out=qt[m][:], in_=psq[:], func=COPY)

            ets = {}

            def emit_scores(j):
                """scoresT + exp for head pair j (full-grid K=128 matmuls
                against zero-padded kt0/kt1)."""
                et0, et1 = [], []
                for s in range(SK):
                    pss0 = ps_s.tile([128, T], F32, tag="pss", name="pss")
                    nc.tensor.matmul(
                        pss0[:], kt0[j][:, s * 128:(s + 1) * 128],
                        qt[j][:, :], start=True, stop=True)
                    pss1 = ps_s.tile([128, T], F32, tag="pss", name="pss")
                    nc.tensor.matmul(
                        pss1[:], kt1[j][:, s * 128:(s + 1) * 128],
                        qt[j][:, :], start=True, stop=True)
                    e0 = p_et.tile([128, T], BF16, tag="et", name="et")
                    nc.scalar.activation(out=e0[:], in_=pss0[:], func=EXP,
                                         scale=float(scale))
                    e1 = p_et.tile([128, T], BF16, tag="et", name="et")
                    nc.scalar.activation(out=e1[:], in_=pss1[:], func=EXP,
                                         scale=float(scale))
                    et0.append(e0)
                    et1.append(e1)
                ets[j] = (et0, et1)

            for m in range(MK):
                for off, sz in nchunks(S):
                    psk = ps_qk.tile([128, sz], F32, tag="psqk", name="psk")
                    for k in range(CK):
                        nc.tensor.matmul(
                            psk[:], wk_sb[k][:, m * 128:(m + 1) * 128],
                            yt[k][:, off:off + sz],
                            start=(k == 0), stop=(k == CK - 1))
                    nc.vector.tensor_copy(kt0[m][0:64, off:off + sz],
                                          psk[0:64, :])
                    nc.vector.tensor_copy(kt1[m][64:128, off:off + sz],
                                          psk[64:128, :])
                emit_scores(m)
            if debug_taps:
                nc.sync.dma_start(out=taps["xt0_d"][:, :], in_=xt[0][:])
            s_wq.close()
            s_xt.close()
            s_wk.close()
            s_psqk.close()

            # ---- phase C: v natural + ones column -------------------------
            ps_v = s_psv.enter_context(
                tc.tile_pool(name="ps_v", bufs=4, space="PSUM"))
            for s in range(SK):
                for off, sz in nchunks(HD):
                    psv = ps_v.tile([128, sz], F32, tag="psv", name="psv")
                    for k in range(CK):
                        nc.tensor.matmul(
                            psv[:], yt[k][:, s * 128:(s + 1) * 128],
                            wv_sb[k][:, off:off + sz],
                            start=(k == 0), stop=(k == CK - 1))
                    h0 = off // 64
                    nc.vector.tensor_copy(
                        v_sb[s][:, h0:h0 + sz // 64, 0:64],
                        psv[:].rearrange("p (h d) -> p h d", d=64))
            s_wv.close()
            s_yt.close()
            s_psv.close()

            # ---- phase D: attention per head pair -------------------------
            ps_av = s_psav.enter_context(
                tc.tile_pool(name="ps_av", bufs=1, space="PSUM"))

            with ExitStack() as ctx_d:
                p_at = ctx_d.enter_context(tc.tile_pool(name="atpool", bufs=1))
                at = [p_at.tile([128, T], BF16, tag=f"at{m}", name=f"at{m}")
                      for m in range(MK)]
                p_pair = ctx_d.enter_context(
                    tc.tile_pool(name="pairpool", bufs=4))
                pairbuf = {}

                def emit_av(j):
                    """AV for pair j — 16 back-to-back full-row matmuls; DVE
                    stages AV rows + denominator rows (partitions 0/32 of a
                    per-pair tile — matmul-legal bases) out of PSUM."""
                    et0, et1 = ets[j]
                    denb = p_norm.tile([33, T], F32, tag="den", bufs=3,
                                       name="denb")
                    pairT = p_pair.tile([128, T], F32, tag="pair", bufs=4,
                                        name="pairT")
                    psav0 = ps_av.tile([65, T], F32, tag="psav", bufs=2,
                                       name="psav")
                    for s in range(SK):
                        nc.tensor.matmul(psav0[:], v_sb[s][:, 2 * j, 0:65],
                                         et0[s][:],
                                         start=(s == 0), stop=(s == SK - 1))
                    nc.vector.tensor_copy(pairT[0:64, :], psav0[0:64, :])
                    nc.vector.tensor_copy(denb[0:1, :], psav0[64:65, :])
                    psav1 = ps_av.tile([65, T], F32, tag="psav", bufs=2,
                                       name="psav")
                    for s in range(SK):
                        nc.tensor.matmul(psav1[:], v_sb[s][:, 2 * j + 1, 0:65],
                                         et1[s][:],
                                         start=(s == 0), stop=(s == SK - 1))
                    nc.vector.tensor_copy(pairT[64:128, :], psav1[0:64, :])
                    nc.vector.tensor_copy(denb[32:33, :], psav1[64:65, :])
                    pairbuf[j] = (pairT, denb)

                def emit_norm(j):
                    """Normalize pair j: one fast reciprocal + one bf16 cast
                    batched over both denominator rows, two K=1 broadcast
                    matmuls, one [128,T] multiply."""
                    pairT, denb = pairbuf.pop(j)
                    recf = p_norm.tile([33, T], F32, tag="recf", bufs=2,
                                       name="recf")
                    rec = p_norm.tile([33, T], BF16, tag="rec", bufs=2,
                                      name="rec")
                    # batched over both parked rows; the unwritten rows in
                    # between produce garbage that is never read
                    nc.vector.reciprocal_approx_fast(
                        out=recf[0:33, :], in_=denb[0:33, :])
                    nc.vector.tensor_copy(rec[0:33, :], recf[0:33, :])
                    psb = ps_av.tile([128, T], F32, tag="psb", bufs=2,
                                     name="psb")
                    nc.tensor.matmul(psb[0:64, :], ones1[0:1, 0:64],
                                     rec[0:1, :], start=True, stop=True)
                    nc.tensor.matmul(psb[64:128, :], ones1[32:33, 0:64],
                                     rec[32:33, :], start=True, stop=True)
                    with nc.allow_low_precision(reason="at is bf16"):
                        nc.vector.tensor_mul(at[j][:], pairT[:], psb[:])

                for j in range(H // 2):
                    emit_av(j)
                    if j >= 1:
                        emit_norm(j - 1)
                emit_norm(H // 2 - 1)

                if debug_taps:
                    nc.sync.dma_start(out=taps["qt0_d"][:, :], in_=qt[0][:])
                    nc.sync.dma_start(out=taps["kt00_d"][:, :], in_=kt0[0][:])
                    nc.sync.dma_start(
                        out=taps["v0_d"][:, :],
                        in_=v_sb[0][:].rearrange("p h d -> p (h d)"))
                    nc.sync.dma_start(out=taps["at0_d"][:, :], in_=at[0][:])

                # ---- phase E: output projection + bias --------------------
                s_pss.close()
                ps_o = ctx_d.enter_context(
                    tc.tile_pool(name="ps_o", bufs=2, space="PSUM",
                                 side="right"))
                p_o = ctx_d.enter_context(tc.tile_pool(name="opool", bufs=2))
                for t_ in range(TK):
                    o_sb = p_o.tile([128, C], F32, tag="osb", name="o_sb")
                    for off, sz in nchunks(C):
                        pso = ps_o.tile([128, sz], F32, tag="pso", name="pso")
                        for mk in range(MK):
                            nc.tensor.matmul(
                                pso[:], at[mk][:, t_ * 128:(t_ + 1) * 128],
                                wot_sb[mk][:, off:off + sz],
                                start=(mk == 0), stop=False)
                        nc.tensor.matmul(pso[:], ones1[0:1, 0:128],
                                         bo_sb[0:1, off:off + sz],
                                         start=False, stop=True)
                        nc.scalar.activation(out=o_sb[:, off:off + sz],
                                             in_=pso[:], func=COPY)
                    for off, sz in nchunks(C):
                        nc.sync.dma_start(
                            out=o[t_ * 128:(t_ + 1) * 128, off:off + sz],
                            in_=o_sb[:, off:off + sz])
            s_wot.close()
            s_et.close()
        finally:
            for st in (s_wq, s_xt, s_wk, s_yt, s_wv, s_wot, s_et,
                       s_psqk, s_pss, s_psv, s_psav):
                st.close()

    nc.compile()
    return nc


# ---------------------------------------------------------------------------
# Host-side wrapper
# ---------------------------------------------------------------------------

_NC_CACHE = {}


def _get_nc():
    key = "full"
    if key not in _NC_CACHE:
        _NC_CACHE[key] = build_attention_nc(
            T=T_FULL * B_FULL // N_CORES, S=S_FULL, C=C_FULL, H=H_FULL,
            D=D_FULL, scale=C_FULL ** (-0.5))
    return _NC_CACHE[key]


def make_in_maps(x, y_enc, Wq, Wk, Wv, Wo, bo):
    """Shard full inputs into the 8 per-core input maps (host prep: bf16
    cast + pre-transpose of x/y, head-stacked weight layouts)."""
    x = np.asarray(x, dtype=np.float32)
    y_enc = np.asarray(y_enc, dtype=np.float32)
    Wq = np.asarray(Wq, dtype=np.float32)
    Wk = np.asarray(Wk, dtype=np.float32)
    Wv = np.asarray(Wv, dtype=np.float32)
    Wo = np.asarray(Wo, dtype=np.float32)
    bo = np.asarray(bo, dtype=np.float32)

    C = Wq.shape[1]
    HD = Wq.shape[0] * Wq.shape[2]
    wq_p = np.ascontiguousarray(
        Wq.transpose(1, 0, 2).reshape(C, HD).astype(BF16_NP))
    wk_p = np.ascontiguousarray(
        Wk.transpose(1, 0, 2).reshape(C, HD).astype(BF16_NP))
    wv_p = np.ascontiguousarray(
        Wv.transpose(1, 0, 2).reshape(C, HD).astype(BF16_NP))
    wot = np.ascontiguousarray(Wo.T.astype(BF16_NP))
    bo2 = np.ascontiguousarray(bo.reshape(1, -1).astype(BF16_NP))

    T = x.shape[1] * x.shape[0] // N_CORES  # rows per core
    yts = [np.ascontiguousarray(y_enc[b].T.astype(BF16_NP))
           for b in range(x.shape[0])]
    in_maps = []
    for core in range(N_CORES):
        b, half = divmod(core, N_CORES // x.shape[0])
        xt = np.ascontiguousarray(
            x[b, half * T:(half + 1) * T].T.astype(BF16_NP))
        in_maps.append({
            "xt": xt, "yt": yts[b],
            "wq": wq_p, "wk": wk_p, "wv": wv_p, "wot": wot, "bo": bo2,
        })
    return in_maps


def run(inputs, trace=False, trace_cores=None):
    """Compile + run on the 8 cores; returns (out, BassKernelResults)."""
    nc = _get_nc()
    in_maps = make_in_maps(**inputs)
    kw = {}
    if trace:
        kw = dict(trace=True,
                  trace_cores=trace_cores if trace_cores is not None else [0])
    res = run_bass_kernel_spmd(nc, in_maps, core_ids=list(range(N_CORES)), **kw)

    B, T_full, C = np.asarray(inputs["x"]).shape
    T = T_full * B // N_CORES
    out = np.empty((B, T_full, C), dtype=np.float32)
    for core in range(N_CORES):
        b, half = divmod(core, N_CORES // B)
        out[b, half * T:(half + 1) * T] = res.results[core]["o"]
    return out, res


def kernel(x, y_enc, Wq, Wk, Wv, Wo, bo):
    out, _ = run(dict(x=x, y_enc=y_enc, Wq=Wq, Wk=Wk, Wv=Wv, Wo=Wo, bo=bo))
    return out
